# revision 38
# baseline (speedup 1.0000x reference)
"""Trainium2 Bass kernel for nn_DepthGlobalPool (histogram_binning).

Math: out[b,:,h,w] = means[bin(b,h,w)] where
  bin = histogram bin of depth over global [min,max], 10 equal bins
  means[n] = mean over pixels p in bin n of (W @ features[p] + bias)
Because the 1x1 conv is linear, the per-bin sums of conv outputs equal
W @ (per-bin sums of features) + count*bias, so the per-pixel conv never
needs to be materialized:
  G[n, o]  = sum_{p in bin n} (W @ features[p])[o]      (device, phase A)
  means    = G_global / max(counts,1) + bias*(counts>0) (host, tiny)
  out[p]   = means[bin(p)]                              (device, phase B)

Distribution: data-parallel over batch B (2 batches per core on 8 cores).
Phase A produces per-core partial G [10,64]; the tiny partials are reduced
on host between the two NEFF launches (cheaper + more deterministic than an
on-device AllReduce, which measured 35-70us of latency+skew).

Phase A (per core): for each 128-pixel block, matmul with the feature block
as the STATIONARY operand (lhsT=[128c,128p], rhs=W^T[128c,64]) produces the
conv output transposed, g_T[128p,64], in PSUM -- this puts pixels on
partitions so a second matmul (lhsT=onehot_T[128p,10], rhs=g_T) can contract
over pixels, accumulating G[10,64] in PSUM across all blocks.

Phase B (per core): out tile [64,512] = means^T @ onehot[10,512] with the
means as stationary; means are split hi/lo into two bf16 matrices so two
accumulating bf16 matmuls reproduce fp32-accurate means (one-hot is exact
in bf16).
"""

import os
import numpy as np
import ml_dtypes

import concourse.bass as bass  # noqa: F401  (registers types)
import concourse.tile as tile
import concourse.bass_utils as bass_utils
from concourse import bacc, mybir

# Problem shape (hardcoded per contract)
B, CIN, COUT, H, W_ = 16, 128, 64, 192, 192
HW = H * W_                      # 36864
NB = 10                          # histogram bins
N_CORES = 8
BPC = B // N_CORES               # batches per core = 2
PPC = BPC * HW                   # pixels per core = 73728
BLK = 128                        # pixels per feature block (matmul stationary)
GROUP_PX = 1024                  # pixels per PSUM group = 8 blocks * 128
BLK_PER_GROUP = GROUP_PX // BLK  # 8
SLAB_PX = 4096                   # pixels per feature DMA slab
N_SLABS = PPC // SLAB_PX         # 18
GROUPS_PER_SLAB = SLAB_PX // GROUP_PX  # 4
N_GROUPS = PPC // GROUP_PX       # 72
N_BLOCKS = PPC // BLK            # 576
OHA_STRIDE = 16                  # onehot_T block stride (padded 10 -> 16
                                 # keeps lhsT slices 32-byte aligned)

BF16 = mybir.dt.bfloat16
F32 = mybir.dt.float32

_CACHE = {}

# exec times (ns) of the last kernel() call, per NEFF, when tracing enabled
LAST_EXEC_NS = {}


def _install_ntff_hook():
    """Optionally enable NTFF profiling under axon (agent image lacks
    antenv.axon_hooks). Best-effort; harmless if unavailable."""
    import sys, types
    if "antenv.axon_hooks" in sys.modules:
        return True
    try:
        mod = types.ModuleType("antenv.axon_hooks")
        _hook = [None]
        mod.set_axon_ntff_profile_hook = lambda h: _hook.__setitem__(0, h)
        mod.get_axon_ntff_profile_hook = lambda: _hook[0]
        import antenv
        from trn_agent_boot.trn_boot import _ntff_profile_via_ctypes
        antenv.axon_hooks = mod
        sys.modules["antenv.axon_hooks"] = mod
        mod.set_axon_ntff_profile_hook(
            _ntff_profile_via_ctypes("/opt/axon/libaxon_pjrt.so"))
        return True
    except Exception:
        return False


def _build_neff_a():
    """Phase A: per-core partial per-bin sums of conv outputs, G[10,64]."""
    nc = bacc.Bacc("TRN2", target_bir_lowering=False, debug=False,
                   enable_asserts=True, num_devices=N_CORES)
    feats_t = nc.dram_tensor("feats", [BPC, CIN, HW], F32, kind="ExternalInput")
    oha_t = nc.dram_tensor("oha", [128, N_BLOCKS * OHA_STRIDE], BF16,
                           kind="ExternalInput")
    wt_t = nc.dram_tensor("wt", [CIN, COUT], BF16, kind="ExternalInput")
    gpart_t = nc.dram_tensor("gpart", [NB, COUT], F32, kind="ExternalOutput")

    feats = feats_t.ap()
    with tile.TileContext(nc) as tc:
        with tc.tile_pool(name="cst", bufs=1) as cst, \
             tc.tile_pool(name="fpool", bufs=3) as fpool, \
             tc.tile_pool(name="gpool", bufs=3) as gpool, \
             tc.tile_pool(name="spool", bufs=1) as spool, \
             tc.tile_pool(name="pconv", bufs=3, space="PSUM") as pconv, \
             tc.tile_pool(name="pwarm", bufs=1, space="PSUM") as pwarm, \
             tc.tile_pool(name="pg", bufs=1, space="PSUM") as pg:

            wt_s = cst.tile([CIN, COUT], BF16)
            nc.sync.dma_start(wt_s[:], wt_t.ap()[:])
            oha_s = cst.tile([128, N_BLOCKS * OHA_STRIDE], BF16)
            nc.sync.dma_start(oha_s[:], oha_t.ap()[:])

            # dependency-free warmup burst: ~5us of dense matmuls trips the
            # PE HAM clock-gate to 2.4 GHz while the first DMAs land
            warm = cst.tile([128, 512], BF16)
            nc.gpsimd.memset(warm[:], 0)
            wps = pwarm.tile([128, 512], F32, space="PSUM")
            for _ in range(12):
                nc.tensor.matmul(wps[:], lhsT=warm[:, :128], rhs=warm[:],
                                 start=True, stop=True)

            G_ps = pg.tile([NB, COUT], F32, space="PSUM")

            gi = 0
            for s in range(N_SLABS):
                px0 = s * SLAB_PX
                b, o = px0 // HW, px0 % HW
                fs = fpool.tile([CIN, SLAB_PX], BF16)
                # SWDGE cast f32 -> bf16 during the DMA
                nc.gpsimd.dma_start(fs[:], feats[b, :, o:o + SLAB_PX])
                for g in range(GROUPS_PER_SLAB):
                    ps = pconv.tile([128, 8 * COUT], F32, space="PSUM")
                    for j in range(BLK_PER_GROUP):
                        f0 = g * GROUP_PX + j * BLK
                        nc.tensor.matmul(
                            ps[:, COUT * j:COUT * (j + 1)],
                            lhsT=fs[:, f0:f0 + BLK],
                            rhs=wt_s[:],
                            start=True, stop=True)
                    gsb = gpool.tile([128, 8 * COUT], BF16)
                    if gi % 2 == 0:
                        nc.vector.tensor_copy(gsb[:], ps[:])
                    else:
                        nc.scalar.copy(gsb[:], ps[:])
                    for j in range(BLK_PER_GROUP):
                        blk = gi * BLK_PER_GROUP + j
                        nc.tensor.matmul(
                            G_ps[:],
                            lhsT=oha_s[:, blk * OHA_STRIDE:blk * OHA_STRIDE + NB],
                            rhs=gsb[:, COUT * j:COUT * (j + 1)],
                            start=(blk == 0), stop=(blk == N_BLOCKS - 1))
                    gi += 1

            g_out = spool.tile([NB, COUT], F32)
            nc.vector.tensor_copy(g_out[:], G_ps[:])
            nc.sync.dma_start(gpart_t.ap()[:], g_out[:])
    nc.compile()
    return nc


def _build_neff_b():
    """Phase B: out[b,:,p] = means[bin(p)] via a means-stationary matmul.

    The hi/lo bf16 split of means is fused into ONE K=20 matmul per 512-px
    chunk: stationary [mh; ml] [20,64], one-hot rows duplicated for the lo
    half, PSUM accumulates both products in fp32.

    DMA-width tricks (both streams must use all 128 partitions to get
    full HBM bandwidth):
      * one-hot is packed [128, PPC/4]: partition rows 32g..32g+20 hold the
        (duplicated) one-hot of the g-th QUARTER of this core's pixels.
        The stationary is replicated at partitions 32g too, since matmul
        requires lhsT/rhs to share a base partition (explicit
        tile_position=(32g, colbase)).
      * output is staged in SBUF as [128=(half,chan), 4608] per 9216-pixel
        slab and written with one 2.36 MB SWDGE DMA (many small sync-ring
        DMAs serialize on one HWDGE queue at ~1/8 bandwidth).
    """
    nc = bacc.Bacc("TRN2", target_bir_lowering=False, debug=False,
                   enable_asserts=True, num_devices=N_CORES)
    mhl_t = nc.dram_tensor("mhl", [128, 4 * COUT], BF16, kind="ExternalInput")
    ohb_t = nc.dram_tensor("ohb", [80, PPC // 4], BF16, kind="ExternalInput")
    # output in half-interleaved layout: out[b, i*64+c, p2] = pixel i*HW2+p2
    # of channel c (host undoes this with one strided copy). This makes the
    # staged write a UNIFORM 2-D [128, 4608] DMA -- measured ~420 GB/s vs
    # ~130 GB/s for the 3-level strided AP of the natural layout.
    HW2 = HW // 2
    out_t = nc.dram_tensor("out", [BPC, 128, HW2], F32, kind="ExternalOutput")

    SLAB = 4608                  # p2-columns per slab
    N_CH = SLAB // 512           # 9 psum chunks per slab
    QUARTER = PPC // 4           # 18432 = pixels per one-hot quarter = HW2

    out_ap = out_t.ap()
    ohb = ohb_t.ap()
    with tile.TileContext(nc) as tc:
        with tc.tile_pool(name="cst", bufs=1) as cst, \
             tc.tile_pool(name="stage", bufs=3) as stage, \
             tc.tile_pool(name="pwarm", bufs=1, space="PSUM") as pwarm, \
             tc.tile_pool(name="pout", bufs=6, space="PSUM") as pout:

            mhl_s = cst.tile([128, 4 * COUT], BF16)
            nc.sync.dma_start(mhl_s[:], mhl_t.ap()[:])

            # warmup burst for the PE HAM clock-gate (overlaps input DMAs)
            warm = cst.tile([128, 512], BF16)
            nc.gpsimd.memset(warm[:], 0)
            wps = pwarm.tile([128, 512], F32, space="PSUM")
            for _ in range(12):
                nc.tensor.matmul(wps[:], lhsT=warm[:, :128], rhs=warm[:],
                                 start=True, stop=True)

            # one-hot double buffer: only rows 0-79 carry data (4 quarters x
            # 20 hi/lo rows); rows 80-127 feed zero weights and just need to
            # be FINITE, so memset them once instead of shipping pad bytes
            oh_buf0 = cst.tile([128, SLAB], BF16)
            oh_buf1 = cst.tile([128, SLAB], BF16)
            oh_bufs = [oh_buf0, oh_buf1]
            for t in oh_bufs:
                # whole-tile memset: rows 80-127 must be FINITE (they feed
                # zero weights). Full-range cover also guarantees Tile
                # orders the per-slab DMAs (rows 0-79) after it.
                nc.gpsimd.memset(t[:], 0)

            ci = 0
            for cs in range(4):      # one-hot column slab: cols [o2, o2+4608)
                o2 = cs * SLAB
                oh_s = oh_bufs[cs % 2]
                nc.sync.dma_start(oh_s[0:80, :], ohb[:, o2:o2 + SLAB])
                for b in range(BPC):
                    # batch b half i lives in one-hot quarter g = 2b+i, and
                    # all four quarters of these columns are already in oh_s
                    sta = stage.tile([128, 4 * 512], F32, tag="sta")
                    stb = stage.tile([128, 5 * 512], F32, tag="stb")
                    for u in range(N_CH):
                        po = pout.tile([128, 512], F32, space="PSUM")
                        rhs = oh_s[:, u * 512:u * 512 + 512]
                        for i, colbase in ((0, 0), (1, 64)):
                            # K=128 block-diagonal stationary: rows 20g..
                            # 20g+20 of column block g=2b+i are the only
                            # nonzeros, so only that quarter's one-hot rows
                            # contribute. (K=128 streams ~379ns/512col;
                            # K=20 measured 625ns.)
                            g = 2 * b + i
                            lhs = mhl_s[:, 64 * g:64 * g + COUT]
                            nc.tensor.matmul(po[colbase:colbase + 64, :],
                                             lhsT=lhs, rhs=rhs,
                                             start=True, stop=True,
                                             tile_position=(0, colbase))
                        st, uu = (sta, u) if u < 4 else (stb, u - 4)
                        if ci % 2 == 0:
                            nc.vector.tensor_copy(st[:, uu * 512:uu * 512 + 512],
                                                  po[:])
                        else:
                            nc.scalar.copy(st[:, uu * 512:uu * 512 + 512], po[:])
                        ci += 1
                        # write each staging half as soon as it completes
                        # (plain 2-D slices of the uniform layout)
                        if u == 3:
                            nc.gpsimd.dma_start(
                                out_ap[b, :, o2:o2 + 2048], sta[:])
                        elif u == N_CH - 1:
                            nc.gpsimd.dma_start(
                                out_ap[b, :, o2 + 2048:o2 + SLAB], stb[:])
    nc.compile()
    return nc


def _get_modules():
    if "a" not in _CACHE:
        _CACHE["a"] = _build_neff_a()
        _CACHE["b"] = _build_neff_b()
    return _CACHE["a"], _CACHE["b"]


def kernel(features, depth, weight, bias, depthpool=None):
    trace = bool(int(os.environ.get("KERNEL_TRACE", "0")))
    if trace:
        trace = _install_ntff_hook()

    features = np.asarray(features, dtype=np.float32)
    depth = np.asarray(depth, dtype=np.float32)
    weight = np.asarray(weight, dtype=np.float32)
    bias = np.asarray(bias, dtype=np.float32)

    # ---- host: histogram binning of depth (exact f32 replica of reference)
    d = depth[:, 0]                                     # [B, H, W] f32
    dmin, dmax = d.min(), d.max()
    width = np.float32((dmax - dmin) / np.float32(NB))
    bins = np.clip(np.floor((d - dmin) / width).astype(np.int32), 0, NB - 1)
    bins = bins.reshape(B, HW)
    counts = np.bincount(bins.ravel(), minlength=NB).astype(np.float64)

    arange_nb = np.arange(NB, dtype=np.int32)
    wt_bf = np.ascontiguousarray(weight.T).astype(ml_dtypes.bfloat16)  # [128,64]

    in_maps_a = []
    in_maps_b_onehot = []
    for c in range(N_CORES):
        binsc = bins[BPC * c:BPC * (c + 1)].reshape(PPC)       # [73728]
        # onehot_T, padded: oha[p, blk*16 + n] = (binsc[blk*128+p] == n)
        bb = binsc.reshape(N_BLOCKS, BLK)                       # [576, 128]
        oha = np.zeros((128, N_BLOCKS, OHA_STRIDE), dtype=ml_dtypes.bfloat16)
        oha[:, :, :NB] = (bb.T[:, :, None] == arange_nb).astype(ml_dtypes.bfloat16)
        oha = np.ascontiguousarray(oha.reshape(128, N_BLOCKS * OHA_STRIDE))
        # one-hot packed [80, PPC/4]: rows 20g+n and 20g+10+n hold
        # (bins[g*QUARTER + j] == n)
        quarter = PPC // 4
        ohb = np.zeros((80, quarter), dtype=ml_dtypes.bfloat16)
        for g in range(4):
            oh1 = (arange_nb[:, None] ==
                   binsc[None, g * quarter:(g + 1) * quarter]
                   ).astype(ml_dtypes.bfloat16)
            ohb[20 * g:20 * g + NB] = oh1
            ohb[20 * g + NB:20 * g + 2 * NB] = oh1
        feats_c = features[BPC * c:BPC * (c + 1)].reshape(BPC, CIN, HW)
        in_maps_a.append({"feats": feats_c, "oha": oha, "wt": wt_bf})
        in_maps_b_onehot.append(ohb)

    nc_a, nc_b = _get_modules()
    core_ids = list(range(N_CORES))

    def _run(nc, in_maps):
        try:
            return bass_utils.run_bass_kernel_spmd(nc, in_maps,
                                                   core_ids=core_ids,
                                                   trace=trace)
        except Exception:
            # one retry for transient device hiccups
            return bass_utils.run_bass_kernel_spmd(nc, in_maps,
                                                   core_ids=core_ids,
                                                   trace=trace)

    res_a = _run(nc_a, in_maps_a)
    if trace:
        LAST_EXEC_NS["A"] = res_a.exec_time_ns

    G = np.zeros((NB, COUT), dtype=np.float64)
    for c in range(N_CORES):
        G += res_a.results[c]["gpart"].astype(np.float64)

    means = G / np.maximum(counts, 1.0)[:, None] \
        + bias.astype(np.float64)[None, :] * (counts > 0)[:, None]
    means = means.astype(np.float32)
    mh = means.astype(ml_dtypes.bfloat16)
    ml = (means - mh.astype(np.float32)).astype(ml_dtypes.bfloat16)
    # block-diagonal stationary: rows 20g..20g+20 nonzero only in column
    # block g (so a full-width K=128 rhs picks out quarter g's one-hot;
    # rows 80-127 are zero to neutralize the unloaded SBUF rows)
    mhl = np.zeros((128, 4 * COUT), dtype=ml_dtypes.bfloat16)
    for g in range(4):
        mhl[20 * g:20 * g + NB, 64 * g:64 * g + COUT] = mh
        mhl[20 * g + NB:20 * g + 2 * NB, 64 * g:64 * g + COUT] = ml

    in_maps_b = [{"mhl": mhl, "ohb": in_maps_b_onehot[c]}
                 for c in range(N_CORES)]
    res_b = _run(nc_b, in_maps_b)
    if trace:
        LAST_EXEC_NS["B"] = res_b.exec_time_ns

    out = np.empty((B, COUT, H, W_), dtype=np.float32)
    for c in range(N_CORES):
        r = res_b.results[c]["out"].reshape(BPC, 2, COUT, HW // 2)
        out[BPC * c:BPC * (c + 1)] = \
            r.transpose(0, 2, 1, 3).reshape(BPC, COUT, H, W_)
    return out


# revision 41
# speedup vs baseline: 1.0082x; 1.0082x over previous
"""Trainium2 Bass kernel for nn_DepthGlobalPool (histogram_binning).

Math: out[b,:,h,w] = means[bin(b,h,w)] where
  bin = histogram bin of depth over global [min,max], 10 equal bins
  means[n] = mean over pixels p in bin n of (W @ features[p] + bias)
Because the 1x1 conv is linear, the per-bin sums of conv outputs equal
W @ (per-bin sums of features) + count*bias, so the per-pixel conv never
needs to be materialized:
  G[n, o]  = sum_{p in bin n} (W @ features[p])[o]      (device, phase A)
  means    = G_global / max(counts,1) + bias*(counts>0) (host, tiny)
  out[p]   = means[bin(p)]                              (device, phase B)

Distribution: data-parallel over batch B (2 batches per core on 8 cores).
Phase A produces per-core partial G [10,64]; the tiny partials are reduced
on host between the two NEFF launches (cheaper + more deterministic than an
on-device AllReduce, which measured 35-70us of latency+skew).

Phase A (per core): for each 128-pixel block, matmul with the feature block
as the STATIONARY operand (lhsT=[128c,128p], rhs=W^T[128c,64]) produces the
conv output transposed, g_T[128p,64], in PSUM -- this puts pixels on
partitions so a second matmul (lhsT=onehot_T[128p,10], rhs=g_T) can contract
over pixels, accumulating G[10,64] in PSUM across all blocks.

Phase B (per core): out tile [64,512] = means^T @ onehot per 512-px chunk,
with the means as the stationary operand. The means are split hi/lo into
two bf16 blocks (one-hot is exact in bf16, PSUM accumulates fp32, so the
output reproduces fp32-accurate means). The hi/lo pair, and the four
"pixel quarter" groups of the packed one-hot, are all fused into a single
K=128 matmul via a block-diagonal stationary (K=128 streams ~1.6x faster
than small K, and a [80,*] one-hot DMA plus zero-weight padding rows beats
shipping pad bytes).
"""

import os
import numpy as np
import ml_dtypes

import concourse.bass as bass  # noqa: F401  (registers types)
import concourse.tile as tile
import concourse.bass_utils as bass_utils
from concourse import bacc, mybir

# Problem shape (hardcoded per contract)
B, CIN, COUT, H, W_ = 16, 128, 64, 192, 192
HW = H * W_                      # 36864
NB = 10                          # histogram bins
N_CORES = 8
BPC = B // N_CORES               # batches per core = 2
PPC = BPC * HW                   # pixels per core = 73728
BLK = 128                        # pixels per feature block (matmul stationary)
GROUP_PX = 1024                  # pixels per PSUM group = 8 blocks * 128
BLK_PER_GROUP = GROUP_PX // BLK  # 8
SLAB_PX = 4096                   # pixels per feature DMA slab
N_SLABS = PPC // SLAB_PX         # 18
GROUPS_PER_SLAB = SLAB_PX // GROUP_PX  # 4
N_GROUPS = PPC // GROUP_PX       # 72
N_BLOCKS = PPC // BLK            # 576
OHA_STRIDE = 16                  # onehot_T block stride (padded 10 -> 16
                                 # keeps lhsT slices 32-byte aligned)

BF16 = mybir.dt.bfloat16
F32 = mybir.dt.float32

_CACHE = {}

# exec times (ns) of the last kernel() call, per NEFF, when tracing enabled
LAST_EXEC_NS = {}


def _install_ntff_hook():
    """Optionally enable NTFF profiling under axon (agent image lacks
    antenv.axon_hooks). Best-effort; harmless if unavailable."""
    import sys, types
    if "antenv.axon_hooks" in sys.modules:
        return True
    try:
        mod = types.ModuleType("antenv.axon_hooks")
        _hook = [None]
        mod.set_axon_ntff_profile_hook = lambda h: _hook.__setitem__(0, h)
        mod.get_axon_ntff_profile_hook = lambda: _hook[0]
        import antenv
        from trn_agent_boot.trn_boot import _ntff_profile_via_ctypes
        antenv.axon_hooks = mod
        sys.modules["antenv.axon_hooks"] = mod
        mod.set_axon_ntff_profile_hook(
            _ntff_profile_via_ctypes("/opt/axon/libaxon_pjrt.so"))
        return True
    except Exception:
        return False


def _build_neff_a():
    """Phase A: per-core partial per-bin sums of conv outputs, G[10,64]."""
    nc = bacc.Bacc("TRN2", target_bir_lowering=False, debug=False,
                   enable_asserts=True, num_devices=N_CORES)
    feats_t = nc.dram_tensor("feats", [BPC, CIN, HW], F32, kind="ExternalInput")
    oha_t = nc.dram_tensor("oha", [128, N_BLOCKS * OHA_STRIDE], BF16,
                           kind="ExternalInput")
    wt_t = nc.dram_tensor("wt", [CIN, COUT], BF16, kind="ExternalInput")
    gpart_t = nc.dram_tensor("gpart", [NB, COUT], F32, kind="ExternalOutput")

    feats = feats_t.ap()
    with tile.TileContext(nc) as tc:
        with tc.tile_pool(name="cst", bufs=1) as cst, \
             tc.tile_pool(name="fpool", bufs=3) as fpool, \
             tc.tile_pool(name="gpool", bufs=3) as gpool, \
             tc.tile_pool(name="spool", bufs=1) as spool, \
             tc.tile_pool(name="pconv", bufs=3, space="PSUM") as pconv, \
             tc.tile_pool(name="pwarm", bufs=1, space="PSUM") as pwarm, \
             tc.tile_pool(name="pg", bufs=1, space="PSUM") as pg:

            wt_s = cst.tile([CIN, COUT], BF16)
            nc.sync.dma_start(wt_s[:], wt_t.ap()[:])
            # onehot_T is DMA'd per-slab inside the loop: one big transfer
            # completes late under contention with the feats stream and all
            # G-matmuls would gate on it (measured as a ~15us slow mode)
            oha_s = cst.tile([128, N_BLOCKS * OHA_STRIDE], BF16)

            # dependency-free warmup burst: ~5us of dense matmuls trips the
            # PE HAM clock-gate to 2.4 GHz while the first DMAs land
            warm = cst.tile([128, 512], BF16)
            nc.gpsimd.memset(warm[:], 0)
            wps = pwarm.tile([128, 512], F32, space="PSUM")
            for _ in range(12):
                nc.tensor.matmul(wps[:], lhsT=warm[:, :128], rhs=warm[:],
                                 start=True, stop=True)

            G_ps = pg.tile([NB, COUT], F32, space="PSUM")

            oha = oha_t.ap()
            blocks_per_slab = SLAB_PX // BLK
            gi = 0
            for s in range(N_SLABS):
                px0 = s * SLAB_PX
                b, o = px0 // HW, px0 % HW
                c0 = s * blocks_per_slab * OHA_STRIDE
                c1 = (s + 1) * blocks_per_slab * OHA_STRIDE
                nc.sync.dma_start(oha_s[:, c0:c1], oha[:, c0:c1])
                fs = fpool.tile([CIN, SLAB_PX], BF16)
                # SWDGE cast f32 -> bf16 during the DMA
                nc.gpsimd.dma_start(fs[:], feats[b, :, o:o + SLAB_PX])
                for g in range(GROUPS_PER_SLAB):
                    ps = pconv.tile([128, 8 * COUT], F32, space="PSUM")
                    for j in range(BLK_PER_GROUP):
                        f0 = g * GROUP_PX + j * BLK
                        nc.tensor.matmul(
                            ps[:, COUT * j:COUT * (j + 1)],
                            lhsT=fs[:, f0:f0 + BLK],
                            rhs=wt_s[:],
                            start=True, stop=True)
                    gsb = gpool.tile([128, 8 * COUT], BF16)
                    if gi % 2 == 0:
                        nc.vector.tensor_copy(gsb[:], ps[:])
                    else:
                        nc.scalar.copy(gsb[:], ps[:])
                    for j in range(BLK_PER_GROUP):
                        blk = gi * BLK_PER_GROUP + j
                        nc.tensor.matmul(
                            G_ps[:],
                            lhsT=oha_s[:, blk * OHA_STRIDE:blk * OHA_STRIDE + NB],
                            rhs=gsb[:, COUT * j:COUT * (j + 1)],
                            start=(blk == 0), stop=(blk == N_BLOCKS - 1))
                    gi += 1

            g_out = spool.tile([NB, COUT], F32)
            nc.vector.tensor_copy(g_out[:], G_ps[:])
            nc.sync.dma_start(gpart_t.ap()[:], g_out[:])
    nc.compile()
    return nc


def _build_neff_b():
    """Phase B: out[b,:,p] = means[bin(p)] via a means-stationary matmul.

    The hi/lo bf16 split of means is fused into ONE K=20 matmul per 512-px
    chunk: stationary [mh; ml] [20,64], one-hot rows duplicated for the lo
    half, PSUM accumulates both products in fp32.

    DMA-width tricks (both streams must use all 128 partitions to get
    full HBM bandwidth):
      * one-hot is packed [128, PPC/4]: partition rows 32g..32g+20 hold the
        (duplicated) one-hot of the g-th QUARTER of this core's pixels.
        The stationary is replicated at partitions 32g too, since matmul
        requires lhsT/rhs to share a base partition (explicit
        tile_position=(32g, colbase)).
      * output is staged in SBUF as [128=(half,chan), 4608] per 9216-pixel
        slab and written with one 2.36 MB SWDGE DMA (many small sync-ring
        DMAs serialize on one HWDGE queue at ~1/8 bandwidth).
    """
    nc = bacc.Bacc("TRN2", target_bir_lowering=False, debug=False,
                   enable_asserts=True, num_devices=N_CORES)
    mhl_t = nc.dram_tensor("mhl", [128, 4 * COUT], BF16, kind="ExternalInput")
    ohb_t = nc.dram_tensor("ohb", [80, PPC // 4], BF16, kind="ExternalInput")
    # output in half-interleaved layout: out[b, i*64+c, p2] = pixel i*HW2+p2
    # of channel c (host undoes this with one strided copy). This makes the
    # staged write a UNIFORM 2-D [128, 4608] DMA -- measured ~420 GB/s vs
    # ~130 GB/s for the 3-level strided AP of the natural layout.
    HW2 = HW // 2
    out_t = nc.dram_tensor("out", [BPC, 128, HW2], F32, kind="ExternalOutput")

    SLAB = 4608                  # p2-columns per slab
    N_CH = SLAB // 512           # 9 psum chunks per slab
    QUARTER = PPC // 4           # 18432 = pixels per one-hot quarter = HW2

    out_ap = out_t.ap()
    ohb = ohb_t.ap()
    with tile.TileContext(nc) as tc:
        with tc.tile_pool(name="cst", bufs=1) as cst, \
             tc.tile_pool(name="stage", bufs=3) as stage, \
             tc.tile_pool(name="pwarm", bufs=1, space="PSUM") as pwarm, \
             tc.tile_pool(name="pout", bufs=6, space="PSUM") as pout:

            mhl_s = cst.tile([128, 4 * COUT], BF16)
            nc.sync.dma_start(mhl_s[:], mhl_t.ap()[:])

            # warmup burst for the PE HAM clock-gate (overlaps input DMAs)
            warm = cst.tile([128, 512], BF16)
            nc.gpsimd.memset(warm[:], 0)
            wps = pwarm.tile([128, 512], F32, space="PSUM")
            for _ in range(12):
                nc.tensor.matmul(wps[:], lhsT=warm[:, :128], rhs=warm[:],
                                 start=True, stop=True)

            # one-hot double buffer: only rows 0-79 carry data (4 quarters x
            # 20 hi/lo rows); rows 80-127 feed zero weights and just need to
            # be FINITE, so memset them once instead of shipping pad bytes
            oh_buf0 = cst.tile([128, SLAB], BF16)
            oh_buf1 = cst.tile([128, SLAB], BF16)
            oh_bufs = [oh_buf0, oh_buf1]
            for t in oh_bufs:
                # whole-tile memset: rows 80-127 must be FINITE (they feed
                # zero weights). Full-range cover also guarantees Tile
                # orders the per-slab DMAs (rows 0-79) after it.
                nc.gpsimd.memset(t[:], 0)

            ci = 0
            for cs in range(4):      # one-hot column slab: cols [o2, o2+4608)
                o2 = cs * SLAB
                oh_s = oh_bufs[cs % 2]
                nc.sync.dma_start(oh_s[0:80, :], ohb[:, o2:o2 + SLAB])
                for b in range(BPC):
                    # batch b half i lives in one-hot quarter g = 2b+i, and
                    # all four quarters of these columns are already in oh_s
                    sta = stage.tile([128, 4 * 512], F32, tag="sta")
                    stb = stage.tile([128, 5 * 512], F32, tag="stb")
                    for u in range(N_CH):
                        po = pout.tile([128, 512], F32, space="PSUM")
                        rhs = oh_s[:, u * 512:u * 512 + 512]
                        for i, colbase in ((0, 0), (1, 64)):
                            # K=128 block-diagonal stationary: rows 20g..
                            # 20g+20 of column block g=2b+i are the only
                            # nonzeros, so only that quarter's one-hot rows
                            # contribute. (K=128 streams ~379ns/512col;
                            # K=20 measured 625ns.)
                            g = 2 * b + i
                            lhs = mhl_s[:, 64 * g:64 * g + COUT]
                            nc.tensor.matmul(po[colbase:colbase + 64, :],
                                             lhsT=lhs, rhs=rhs,
                                             start=True, stop=True,
                                             tile_position=(0, colbase))
                        st, uu = (sta, u) if u < 4 else (stb, u - 4)
                        if ci % 2 == 0:
                            nc.vector.tensor_copy(st[:, uu * 512:uu * 512 + 512],
                                                  po[:])
                        else:
                            nc.scalar.copy(st[:, uu * 512:uu * 512 + 512], po[:])
                        ci += 1
                        # write each staging half as soon as it completes
                        # (plain 2-D slices of the uniform layout)
                        if u == 3:
                            nc.gpsimd.dma_start(
                                out_ap[b, :, o2:o2 + 2048], sta[:])
                        elif u == N_CH - 1:
                            nc.gpsimd.dma_start(
                                out_ap[b, :, o2 + 2048:o2 + SLAB], stb[:])
    nc.compile()
    return nc


def _get_modules():
    if "a" not in _CACHE:
        _CACHE["a"] = _build_neff_a()
        _CACHE["b"] = _build_neff_b()
    return _CACHE["a"], _CACHE["b"]


def kernel(features, depth, weight, bias, depthpool=None):
    trace = bool(int(os.environ.get("KERNEL_TRACE", "0")))
    if trace:
        trace = _install_ntff_hook()

    features = np.asarray(features, dtype=np.float32)
    depth = np.asarray(depth, dtype=np.float32)
    weight = np.asarray(weight, dtype=np.float32)
    bias = np.asarray(bias, dtype=np.float32)

    # ---- host: histogram binning of depth (exact f32 replica of reference)
    d = depth[:, 0]                                     # [B, H, W] f32
    dmin, dmax = d.min(), d.max()
    width = np.float32((dmax - dmin) / np.float32(NB))
    bins = np.clip(np.floor((d - dmin) / width).astype(np.int32), 0, NB - 1)
    bins = bins.reshape(B, HW)
    counts = np.bincount(bins.ravel(), minlength=NB).astype(np.float64)

    arange_nb = np.arange(NB, dtype=np.int32)
    wt_bf = np.ascontiguousarray(weight.T).astype(ml_dtypes.bfloat16)  # [128,64]

    in_maps_a = []
    in_maps_b_onehot = []
    for c in range(N_CORES):
        binsc = bins[BPC * c:BPC * (c + 1)].reshape(PPC)       # [73728]
        # onehot_T, padded: oha[p, blk*16 + n] = (binsc[blk*128+p] == n)
        bb = binsc.reshape(N_BLOCKS, BLK)                       # [576, 128]
        oha = np.zeros((128, N_BLOCKS, OHA_STRIDE), dtype=ml_dtypes.bfloat16)
        oha[:, :, :NB] = (bb.T[:, :, None] == arange_nb).astype(ml_dtypes.bfloat16)
        oha = np.ascontiguousarray(oha.reshape(128, N_BLOCKS * OHA_STRIDE))
        # one-hot packed [80, PPC/4]: rows 20g+n and 20g+10+n hold
        # (bins[g*QUARTER + j] == n)
        quarter = PPC // 4
        ohb = np.zeros((80, quarter), dtype=ml_dtypes.bfloat16)
        for g in range(4):
            oh1 = (arange_nb[:, None] ==
                   binsc[None, g * quarter:(g + 1) * quarter]
                   ).astype(ml_dtypes.bfloat16)
            ohb[20 * g:20 * g + NB] = oh1
            ohb[20 * g + NB:20 * g + 2 * NB] = oh1
        feats_c = features[BPC * c:BPC * (c + 1)].reshape(BPC, CIN, HW)
        in_maps_a.append({"feats": feats_c, "oha": oha, "wt": wt_bf})
        in_maps_b_onehot.append(ohb)

    nc_a, nc_b = _get_modules()
    core_ids = list(range(N_CORES))

    def _run(nc, in_maps):
        try:
            return bass_utils.run_bass_kernel_spmd(nc, in_maps,
                                                   core_ids=core_ids,
                                                   trace=trace)
        except Exception:
            # one retry for transient device hiccups
            return bass_utils.run_bass_kernel_spmd(nc, in_maps,
                                                   core_ids=core_ids,
                                                   trace=trace)

    res_a = _run(nc_a, in_maps_a)
    if trace:
        LAST_EXEC_NS["A"] = res_a.exec_time_ns

    G = np.zeros((NB, COUT), dtype=np.float64)
    for c in range(N_CORES):
        G += res_a.results[c]["gpart"].astype(np.float64)

    means = G / np.maximum(counts, 1.0)[:, None] \
        + bias.astype(np.float64)[None, :] * (counts > 0)[:, None]
    means = means.astype(np.float32)
    mh = means.astype(ml_dtypes.bfloat16)
    ml = (means - mh.astype(np.float32)).astype(ml_dtypes.bfloat16)
    # block-diagonal stationary: rows 20g..20g+20 nonzero only in column
    # block g (so a full-width K=128 rhs picks out quarter g's one-hot;
    # rows 80-127 are zero to neutralize the unloaded SBUF rows)
    mhl = np.zeros((128, 4 * COUT), dtype=ml_dtypes.bfloat16)
    for g in range(4):
        mhl[20 * g:20 * g + NB, 64 * g:64 * g + COUT] = mh
        mhl[20 * g + NB:20 * g + 2 * NB, 64 * g:64 * g + COUT] = ml

    in_maps_b = [{"mhl": mhl, "ohb": in_maps_b_onehot[c]}
                 for c in range(N_CORES)]
    res_b = _run(nc_b, in_maps_b)
    if trace:
        LAST_EXEC_NS["B"] = res_b.exec_time_ns

    out = np.empty((B, COUT, H, W_), dtype=np.float32)
    for c in range(N_CORES):
        r = res_b.results[c]["out"].reshape(BPC, 2, COUT, HW // 2)
        out[BPC * c:BPC * (c + 1)] = \
            r.transpose(0, 2, 1, 3).reshape(BPC, COUT, H, W_)
    return out


# revision 43
# speedup vs baseline: 1.0274x; 1.0190x over previous
"""Trainium2 Bass kernel for nn_DepthGlobalPool (histogram_binning).

Math: out[b,:,h,w] = means[bin(b,h,w)] where
  bin = histogram bin of depth over global [min,max], 10 equal bins
  means[n] = mean over pixels p in bin n of (W @ features[p] + bias)
Because the 1x1 conv is linear, the per-bin sums of conv outputs equal
W @ (per-bin sums of features) + count*bias, so the per-pixel conv never
needs to be materialized:
  G[n, o]  = sum_{p in bin n} (W @ features[p])[o]      (device, phase A)
  means    = G_global / max(counts,1) + bias*(counts>0) (host, tiny)
  out[p]   = means[bin(p)]                              (device, phase B)

Distribution: data-parallel over batch B (2 batches per core on 8 cores).
Phase A produces per-core partial G [10,64]; the tiny partials are reduced
on host between the two NEFF launches (cheaper + more deterministic than an
on-device AllReduce, which measured 35-70us of latency+skew).

Phase A (per core): for each 128-pixel block, matmul with the feature block
as the STATIONARY operand (lhsT=[128c,128p], rhs=W^T[128c,64]) produces the
conv output transposed, g_T[128p,64], in PSUM -- this puts pixels on
partitions so a second matmul (lhsT=onehot_T[128p,10], rhs=g_T) can contract
over pixels, accumulating G[10,64] in PSUM across all blocks.

Phase B (per core): out tile [64,512] = means^T @ onehot per 512-px chunk,
with the means as the stationary operand. The means are split hi/lo into
two bf16 blocks (one-hot is exact in bf16, PSUM accumulates fp32, so the
output reproduces fp32-accurate means). The hi/lo pair, and the four
"pixel quarter" groups of the packed one-hot, are all fused into a single
K=128 matmul via a block-diagonal stationary (K=128 streams ~1.6x faster
than small K, and a [80,*] one-hot DMA plus zero-weight padding rows beats
shipping pad bytes).
"""

import os
import numpy as np
import ml_dtypes

import concourse.bass as bass  # noqa: F401  (registers types)
import concourse.tile as tile
import concourse.bass_utils as bass_utils
from concourse import bacc, mybir

# Problem shape (hardcoded per contract)
B, CIN, COUT, H, W_ = 16, 128, 64, 192, 192
HW = H * W_                      # 36864
NB = 10                          # histogram bins
N_CORES = 8
BPC = B // N_CORES               # batches per core = 2
PPC = BPC * HW                   # pixels per core = 73728
BLK = 128                        # pixels per feature block (matmul stationary)
GROUP_PX = 1024                  # pixels per PSUM group = 8 blocks * 128
BLK_PER_GROUP = GROUP_PX // BLK  # 8
SLAB_PX = 4096                   # pixels per feature DMA slab
N_SLABS = PPC // SLAB_PX         # 18
GROUPS_PER_SLAB = SLAB_PX // GROUP_PX  # 4
N_GROUPS = PPC // GROUP_PX       # 72
N_BLOCKS = PPC // BLK            # 576
OHA_STRIDE = 16                  # onehot_T block stride (padded 10 -> 16
                                 # keeps lhsT slices 32-byte aligned)

BF16 = mybir.dt.bfloat16
F32 = mybir.dt.float32

_CACHE = {}

# exec times (ns) of the last kernel() call, per NEFF, when tracing enabled
LAST_EXEC_NS = {}


def _install_ntff_hook():
    """Optionally enable NTFF profiling under axon (agent image lacks
    antenv.axon_hooks). Best-effort; harmless if unavailable."""
    import sys, types
    if "antenv.axon_hooks" in sys.modules:
        return True
    try:
        mod = types.ModuleType("antenv.axon_hooks")
        _hook = [None]
        mod.set_axon_ntff_profile_hook = lambda h: _hook.__setitem__(0, h)
        mod.get_axon_ntff_profile_hook = lambda: _hook[0]
        import antenv
        from trn_agent_boot.trn_boot import _ntff_profile_via_ctypes
        antenv.axon_hooks = mod
        sys.modules["antenv.axon_hooks"] = mod
        mod.set_axon_ntff_profile_hook(
            _ntff_profile_via_ctypes("/opt/axon/libaxon_pjrt.so"))
        return True
    except Exception:
        return False


def _build_neff_a():
    """Phase A: per-core partial per-bin sums of conv outputs, G[10,64]."""
    nc = bacc.Bacc("TRN2", target_bir_lowering=False, debug=False,
                   enable_asserts=True, num_devices=N_CORES)
    feats_t = nc.dram_tensor("feats", [BPC, CIN, HW], F32, kind="ExternalInput")
    oha_t = nc.dram_tensor("oha", [128, N_BLOCKS * OHA_STRIDE], BF16,
                           kind="ExternalInput")
    wt_t = nc.dram_tensor("wt", [CIN, COUT], BF16, kind="ExternalInput")
    gpart_t = nc.dram_tensor("gpart", [NB, COUT], F32, kind="ExternalOutput")

    feats = feats_t.ap()
    with tile.TileContext(nc) as tc:
        with tc.tile_pool(name="cst", bufs=1) as cst, \
             tc.tile_pool(name="fpool", bufs=3) as fpool, \
             tc.tile_pool(name="gpool", bufs=3) as gpool, \
             tc.tile_pool(name="spool", bufs=1) as spool, \
             tc.tile_pool(name="pconv", bufs=3, space="PSUM") as pconv, \
             tc.tile_pool(name="pwarm", bufs=1, space="PSUM") as pwarm, \
             tc.tile_pool(name="pg", bufs=1, space="PSUM") as pg:

            wt_s = cst.tile([CIN, COUT], BF16)
            nc.sync.dma_start(wt_s[:], wt_t.ap()[:])
            # onehot_T is DMA'd per-slab inside the loop: one big transfer
            # completes late under contention with the feats stream and all
            # G-matmuls would gate on it (measured as a ~15us slow mode)
            oha_s = cst.tile([128, N_BLOCKS * OHA_STRIDE], BF16)

            # dependency-free warmup burst: ~5us of dense matmuls trips the
            # PE HAM clock-gate to 2.4 GHz while the first DMAs land
            warm = cst.tile([128, 512], BF16)
            nc.gpsimd.memset(warm[:], 0)
            wps = pwarm.tile([128, 512], F32, space="PSUM")
            for _ in range(12):
                nc.tensor.matmul(wps[:], lhsT=warm[:, :128], rhs=warm[:],
                                 start=True, stop=True)

            G_ps = pg.tile([NB, COUT], F32, space="PSUM")

            oha = oha_t.ap()
            blocks_per_slab = SLAB_PX // BLK
            gi = 0
            for s in range(N_SLABS):
                px0 = s * SLAB_PX
                b, o = px0 // HW, px0 % HW
                c0 = s * blocks_per_slab * OHA_STRIDE
                c1 = (s + 1) * blocks_per_slab * OHA_STRIDE
                nc.sync.dma_start(oha_s[:, c0:c1], oha[:, c0:c1])
                fs = fpool.tile([CIN, SLAB_PX], BF16)
                # SWDGE cast f32 -> bf16 during the DMA
                nc.gpsimd.dma_start(fs[:], feats[b, :, o:o + SLAB_PX])
                for g in range(GROUPS_PER_SLAB):
                    ps = pconv.tile([128, 8 * COUT], F32, space="PSUM")
                    for j in range(BLK_PER_GROUP):
                        f0 = g * GROUP_PX + j * BLK
                        nc.tensor.matmul(
                            ps[:, COUT * j:COUT * (j + 1)],
                            lhsT=fs[:, f0:f0 + BLK],
                            rhs=wt_s[:],
                            start=True, stop=True)
                    gsb = gpool.tile([128, 8 * COUT], BF16)
                    if gi % 2 == 0:
                        nc.vector.tensor_copy(gsb[:], ps[:])
                    else:
                        nc.scalar.copy(gsb[:], ps[:])
                    for j in range(BLK_PER_GROUP):
                        blk = gi * BLK_PER_GROUP + j
                        nc.tensor.matmul(
                            G_ps[:],
                            lhsT=oha_s[:, blk * OHA_STRIDE:blk * OHA_STRIDE + NB],
                            rhs=gsb[:, COUT * j:COUT * (j + 1)],
                            start=(blk == 0), stop=(blk == N_BLOCKS - 1))
                    gi += 1

            g_out = spool.tile([NB, COUT], F32)
            nc.vector.tensor_copy(g_out[:], G_ps[:])
            nc.sync.dma_start(gpart_t.ap()[:], g_out[:])
    nc.compile()
    return nc


def _build_neff_b():
    """Phase B: out[b,:,p] = means[bin(p)] via a means-stationary matmul.

    The hi/lo bf16 split of means is fused into ONE K=20 matmul per 512-px
    chunk: stationary [mh; ml] [20,64], one-hot rows duplicated for the lo
    half, PSUM accumulates both products in fp32.

    DMA-width tricks (both streams must use all 128 partitions to get
    full HBM bandwidth):
      * one-hot is packed [128, PPC/4]: partition rows 32g..32g+20 hold the
        (duplicated) one-hot of the g-th QUARTER of this core's pixels.
        The stationary is replicated at partitions 32g too, since matmul
        requires lhsT/rhs to share a base partition (explicit
        tile_position=(32g, colbase)).
      * output is staged in SBUF as [128=(half,chan), 4608] per 9216-pixel
        slab and written with one 2.36 MB SWDGE DMA (many small sync-ring
        DMAs serialize on one HWDGE queue at ~1/8 bandwidth).
    """
    nc = bacc.Bacc("TRN2", target_bir_lowering=False, debug=False,
                   enable_asserts=True, num_devices=N_CORES)
    mhl_t = nc.dram_tensor("mhl", [128, 4 * COUT], BF16, kind="ExternalInput")
    ohb_t = nc.dram_tensor("ohb", [80, PPC // 4], BF16, kind="ExternalInput")
    # output in half-interleaved layout: out[b, i*64+c, p2] = pixel i*HW2+p2
    # of channel c (host undoes this with one strided copy). This makes the
    # staged write a UNIFORM 2-D [128, 4608] DMA -- measured ~420 GB/s vs
    # ~130 GB/s for the 3-level strided AP of the natural layout.
    HW2 = HW // 2
    out_t = nc.dram_tensor("out", [BPC, 128, HW2], F32, kind="ExternalOutput")

    SLAB = 4608                  # p2-columns per slab
    N_CH = SLAB // 512           # 9 psum chunks per slab
    QUARTER = PPC // 4           # 18432 = pixels per one-hot quarter = HW2

    out_ap = out_t.ap()
    ohb = ohb_t.ap()
    with tile.TileContext(nc) as tc:
        with tc.tile_pool(name="cst", bufs=1) as cst, \
             tc.tile_pool(name="stage", bufs=6) as stage, \
             tc.tile_pool(name="pwarm", bufs=1, space="PSUM") as pwarm, \
             tc.tile_pool(name="pout", bufs=6, space="PSUM") as pout:

            mhl_s = cst.tile([128, 4 * COUT], BF16)
            nc.sync.dma_start(mhl_s[:], mhl_t.ap()[:])

            # warmup burst for the PE HAM clock-gate (overlaps input DMAs)
            warm = cst.tile([128, 512], BF16)
            nc.gpsimd.memset(warm[:], 0)
            wps = pwarm.tile([128, 512], F32, space="PSUM")
            for _ in range(12):
                nc.tensor.matmul(wps[:], lhsT=warm[:, :128], rhs=warm[:],
                                 start=True, stop=True)

            # one-hot double buffer: only rows 0-79 carry data (4 quarters x
            # 20 hi/lo rows); rows 80-127 feed zero weights and just need to
            # be FINITE, so memset them once instead of shipping pad bytes
            oh_buf0 = cst.tile([128, SLAB], BF16)
            oh_buf1 = cst.tile([128, SLAB], BF16)
            oh_bufs = [oh_buf0, oh_buf1]
            for t in oh_bufs:
                # whole-tile memset: rows 80-127 must be FINITE (they feed
                # zero weights). Full-range cover also guarantees Tile
                # orders the per-slab DMAs (rows 0-79) after it.
                nc.gpsimd.memset(t[:], 0)

            ci = 0
            for cs in range(4):      # one-hot column slab: cols [o2, o2+4608)
                o2 = cs * SLAB
                oh_s = oh_bufs[cs % 2]
                nc.sync.dma_start(oh_s[0:80, :], ohb[:, o2:o2 + SLAB])
                for b in range(BPC):
                    # batch b half i lives in one-hot quarter g = 2b+i, and
                    # all four quarters of these columns are already in oh_s
                    # 9 chunks staged as pieces of 2/2/2/3 so each write DMA
                    # departs ~2 chunks after its data exists -- the write
                    # stream is HBM-bound, so starting it earlier shortens
                    # the whole kernel (head offset == tail length)
                    pieces = (2, 2, 2, 3)
                    pc = None
                    pi = 0
                    u0 = 0
                    for u in range(N_CH):
                        po = pout.tile([128, 512], F32, space="PSUM")
                        rhs = oh_s[:, u * 512:u * 512 + 512]
                        for i, colbase in ((0, 0), (1, 64)):
                            # K=128 block-diagonal stationary: rows 20g..
                            # 20g+20 of column block g=2b+i are the only
                            # nonzeros, so only that quarter's one-hot rows
                            # contribute. (K=128 streams ~379ns/512col;
                            # K=20 measured 625ns.)
                            g = 2 * b + i
                            lhs = mhl_s[:, 64 * g:64 * g + COUT]
                            nc.tensor.matmul(po[colbase:colbase + 64, :],
                                             lhsT=lhs, rhs=rhs,
                                             start=True, stop=True,
                                             tile_position=(0, colbase))
                        if pc is None:
                            pc = stage.tile([128, 3 * 512], F32, tag="pc")
                            u0 = u
                        uu = u - u0
                        if ci % 2 == 0:
                            nc.vector.tensor_copy(pc[:, uu * 512:uu * 512 + 512],
                                                  po[:])
                        else:
                            nc.scalar.copy(pc[:, uu * 512:uu * 512 + 512], po[:])
                        ci += 1
                        if uu == pieces[pi] - 1:
                            nsz = pieces[pi] * 512
                            nc.gpsimd.dma_start(
                                out_ap[b, :, o2 + u0 * 512:o2 + u0 * 512 + nsz],
                                pc[:, :nsz])
                            pc = None
                            pi += 1
    nc.compile()
    return nc


def _get_modules():
    if "a" not in _CACHE:
        _CACHE["a"] = _build_neff_a()
        _CACHE["b"] = _build_neff_b()
    return _CACHE["a"], _CACHE["b"]


def kernel(features, depth, weight, bias, depthpool=None):
    trace = bool(int(os.environ.get("KERNEL_TRACE", "0")))
    if trace:
        trace = _install_ntff_hook()

    features = np.asarray(features, dtype=np.float32)
    depth = np.asarray(depth, dtype=np.float32)
    weight = np.asarray(weight, dtype=np.float32)
    bias = np.asarray(bias, dtype=np.float32)

    # ---- host: histogram binning of depth (exact f32 replica of reference)
    d = depth[:, 0]                                     # [B, H, W] f32
    dmin, dmax = d.min(), d.max()
    width = np.float32((dmax - dmin) / np.float32(NB))
    bins = np.clip(np.floor((d - dmin) / width).astype(np.int32), 0, NB - 1)
    bins = bins.reshape(B, HW)
    counts = np.bincount(bins.ravel(), minlength=NB).astype(np.float64)

    arange_nb = np.arange(NB, dtype=np.int32)
    wt_bf = np.ascontiguousarray(weight.T).astype(ml_dtypes.bfloat16)  # [128,64]

    in_maps_a = []
    in_maps_b_onehot = []
    for c in range(N_CORES):
        binsc = bins[BPC * c:BPC * (c + 1)].reshape(PPC)       # [73728]
        # onehot_T, padded: oha[p, blk*16 + n] = (binsc[blk*128+p] == n)
        bb = binsc.reshape(N_BLOCKS, BLK)                       # [576, 128]
        oha = np.zeros((128, N_BLOCKS, OHA_STRIDE), dtype=ml_dtypes.bfloat16)
        oha[:, :, :NB] = (bb.T[:, :, None] == arange_nb).astype(ml_dtypes.bfloat16)
        oha = np.ascontiguousarray(oha.reshape(128, N_BLOCKS * OHA_STRIDE))
        # one-hot packed [80, PPC/4]: rows 20g+n and 20g+10+n hold
        # (bins[g*QUARTER + j] == n)
        quarter = PPC // 4
        ohb = np.zeros((80, quarter), dtype=ml_dtypes.bfloat16)
        for g in range(4):
            oh1 = (arange_nb[:, None] ==
                   binsc[None, g * quarter:(g + 1) * quarter]
                   ).astype(ml_dtypes.bfloat16)
            ohb[20 * g:20 * g + NB] = oh1
            ohb[20 * g + NB:20 * g + 2 * NB] = oh1
        feats_c = features[BPC * c:BPC * (c + 1)].reshape(BPC, CIN, HW)
        in_maps_a.append({"feats": feats_c, "oha": oha, "wt": wt_bf})
        in_maps_b_onehot.append(ohb)

    nc_a, nc_b = _get_modules()
    core_ids = list(range(N_CORES))

    def _run(nc, in_maps):
        try:
            return bass_utils.run_bass_kernel_spmd(nc, in_maps,
                                                   core_ids=core_ids,
                                                   trace=trace)
        except Exception:
            # one retry for transient device hiccups
            return bass_utils.run_bass_kernel_spmd(nc, in_maps,
                                                   core_ids=core_ids,
                                                   trace=trace)

    res_a = _run(nc_a, in_maps_a)
    if trace:
        LAST_EXEC_NS["A"] = res_a.exec_time_ns

    G = np.zeros((NB, COUT), dtype=np.float64)
    for c in range(N_CORES):
        G += res_a.results[c]["gpart"].astype(np.float64)

    means = G / np.maximum(counts, 1.0)[:, None] \
        + bias.astype(np.float64)[None, :] * (counts > 0)[:, None]
    means = means.astype(np.float32)
    mh = means.astype(ml_dtypes.bfloat16)
    ml = (means - mh.astype(np.float32)).astype(ml_dtypes.bfloat16)
    # block-diagonal stationary: rows 20g..20g+20 nonzero only in column
    # block g (so a full-width K=128 rhs picks out quarter g's one-hot;
    # rows 80-127 are zero to neutralize the unloaded SBUF rows)
    mhl = np.zeros((128, 4 * COUT), dtype=ml_dtypes.bfloat16)
    for g in range(4):
        mhl[20 * g:20 * g + NB, 64 * g:64 * g + COUT] = mh
        mhl[20 * g + NB:20 * g + 2 * NB, 64 * g:64 * g + COUT] = ml

    in_maps_b = [{"mhl": mhl, "ohb": in_maps_b_onehot[c]}
                 for c in range(N_CORES)]
    res_b = _run(nc_b, in_maps_b)
    if trace:
        LAST_EXEC_NS["B"] = res_b.exec_time_ns

    out = np.empty((B, COUT, H, W_), dtype=np.float32)
    for c in range(N_CORES):
        r = res_b.results[c]["out"].reshape(BPC, 2, COUT, HW // 2)
        out[BPC * c:BPC * (c + 1)] = \
            r.transpose(0, 2, 1, 3).reshape(BPC, COUT, H, W_)
    return out
